# revision 5
# baseline (speedup 1.0000x reference)
"""Euler characteristic curve (cubical complex) kernel for Trainium2.

Problem: x [32,16,128,128] f32 -> ECC [32,16,64] f32.
Per (b,c) slice: every cell of the 255x255 vertex-mode cubical grid has
filtration bin K = ceil(63*max(corner values)) in [0,63];
ECC(t) = #V(K<=t) - #Eh(K<=t) - #Ev(K<=t) + #Q(K<=t).

Strategy (per core, 64 slices, pure data parallel over 8 cores):
 - Lower-star compression: chi(t) = sum_v w_v * [K_v <= t] with integer
   vertex weight w_v = 1 - (#edges assigned) + (#squares assigned).
 - Min-clamp difference trick (no per-threshold mask*weight multiply):
   encode V = K16 + w/32 (fp16-exact: |K16|<=32 needs 6 bits + 5
   fraction bits, K16 = K - 32). With cut c_t = t - 31.5:
       sum_f [min(V,c_t) - min(K16,c_t)] = (1/32) * sum_v w_v [K_v<=t]
   exactly (K=t kept since V <= K16+1/32 < c_t+...; K=t+1 clamped since
   V >= K16-3/32 > c_t; sentinels clamp on both sides and cancel).
 - Per threshold: one DVE tensor_scalar (min,mult) pass at 4x producing
   zV; the K16 baseline either as a second DVE min pass (PE subtracts it
   via a negated bsel column block accumulated into the same PSUM) or on
   ACT as Relu(K16 - c_t)+accum (sum min = SK0 - sum relu), chosen per
   threshold to balance engines. PE column-sum matmuls with +/-1 slice-
   selector weights, ACT Copy(scale=32)+accum tail -> chi[:, t].
 - chi[:, 63] holds 32*SK0 per slice (t=63 output is the host constant 1).
 - All arithmetic exact in integers/32nds; rel err 0.
"""

import numpy as np

B, C, H, W = 32, 16, 128, 128
RES = 64
NCORES = 8
SLICES = B * C              # 512
SPC = SLICES // NCORES      # 64 slices per core
NPART = 128

SW = 130                    # row stride: 128 cols + 2 sentinel columns
ROWS = 67                   # pad row + up-overlap + 64 owned + down-overlap
WTOT = ROWS * SW            # 8710 input width per partition
KW = WTOT + 4               # K tile width (pad, memset to sentinel)
OWN = 260                   # owned rows start (flat offset, row 2)
OWN_W = 64 * SW             # 8320 owned width
EHX_W = 8582                # ehx width (Eh over flat 129..8711)
XSENT = 20.25               # x sentinel -> K = 1276, K16 = 1244
SENT16 = 1244.0
MCH = 256                   # PE chunk width
NMM = (OWN_W + MCH - 1) // MCH

# thresholds whose K16-baseline runs on ACT (Relu+accum); rest on DVE.
# ~35 of 63 balances ACT vs DVE/PE; first 12 emitted during w-prep.


def _act_ts(k=35):
    """Evenly spread k ACT-baseline thresholds among 0..62."""
    idx = np.linspace(0, 62, k).round().astype(int)
    return tuple(sorted(set(idx.tolist())))


ACT_TS = _act_ts(36)
AF_TS = ACT_TS[2::5][:8]            # ~8 ACT-full thresholds (relu-V too)

_CACHE = {}


def _build_program(legalize=True):
    import concourse.bass as bass
    import concourse.mybir as mybir
    from concourse.tile import TileContext
    from concourse.alu_op_type import AluOpType as alu

    dt = mybir.dt
    af = mybir.ActivationFunctionType
    nc = bass.Bass("TRN2", target_bir_lowering=False, debug=False)

    x_dram = nc.dram_tensor("xi", [NPART, WTOT], dt.float32, kind="ExternalInput").ap()
    bsel_dram = nc.dram_tensor("bsel", [NPART, 128], dt.float32, kind="ExternalInput").ap()
    bias_dram = nc.dram_tensor("bias", [NPART, 64], dt.float32, kind="ExternalInput").ap()
    chi_dram = nc.dram_tensor("chi", [SPC, 64], dt.float32, kind="ExternalOutput").ap()
    akr_dram = nc.dram_tensor("akr", [NPART, 64], dt.float32, kind="ExternalOutput").ap()
    zchi_dram = nc.dram_tensor("zchi", [SPC, 64], dt.float32, kind="ExternalOutput").ap()
    akrv_dram = nc.dram_tensor("akrv", [NPART, 64], dt.float32, kind="ExternalOutput").ap()

    HCH = 2178  # K-compute column chunk width (4 chunks, last 2176)

    act_list = list(ACT_TS)
    af_set = set(AF_TS)
    dve_list = [t for t in range(63) if t not in ACT_TS]

    with TileContext(nc) as tc:
        with (
            tc.tile_pool(name="persist", bufs=1) as ap_,
            tc.tile_pool(name="ps", bufs=8, space="PSUM") as pp,
        ):
            # ---- persistent tiles ----
            K = ap_.tile([NPART, KW], dt.float16)     # K16 = ceil(63x) - 32
            V = ap_.tile([NPART, OWN_W], dt.float16)  # K16 + w/32
            bself = ap_.tile([NPART, 128], dt.float32)
            bselh = ap_.tile([NPART, 128], dt.float16)
            biasT = ap_.tile([NPART, 64], dt.float32)
            chi = ap_.tile([SPC, 64], dt.float32)
            zchi = ap_.tile([SPC, 64], dt.float32)
            akr = ap_.tile([NPART, 64], dt.float32)
            akrv = ap_.tile([NPART, 64], dt.float32)
            scr512 = ap_.tile([SPC, MCH], dt.float32)
            za = ap_.tile([NPART, OWN_W], dt.float8e4)  # ACT relu scratch

            nc.sync.dma_start(bself[:, :], bsel_dram)
            nc.sync.dma_start(biasT[:, :], bias_dram)
            nc.vector.tensor_copy(bselh[:, :], bself[:, :])
            nc.vector.memset(K[:, WTOT:KW], SENT16)
            nc.vector.memset(akr[:, :], 0.0)
            nc.vector.memset(akrv[:, :], 0.0)

            Ko = K[:, OWN : OWN + OWN_W]

            def emit_relu(t):
                nc.scalar.activation(
                    za[:, :], Ko, af.Relu, bias=biasT[:, t : t + 1], scale=1.0,
                    accum_out=akr[:, t : t + 1],
                )

            def emit_relu_v(t):
                nc.scalar.activation(
                    za[:, :], V[:, :], af.Relu, bias=biasT[:, t : t + 1], scale=1.0,
                    accum_out=akrv[:, t : t + 1],
                )

            # ---- K16 = ceil(63*x) - 32 ----
            with tc.tile_pool(name="kprep", bufs=1) as kp:
                xf = kp.tile([NPART, WTOT], dt.float32)
                ft = kp.tile([NPART, 2 * HCH], dt.float32)
                it_ = kp.tile([NPART, HCH], dt.int32)
                ht = kp.tile([NPART, 2 * HCH], dt.float16)
                chunks = [(i * HCH, min((i + 1) * HCH, WTOT)) for i in range(4)]
                for lo, hi in chunks:
                    nc.sync.dma_start(xf[:, lo:hi], x_dram[:, lo:hi])
                for lo, hi in chunks:
                    cw = hi - lo
                    y = ft[:, 0:cw]
                    yt = ft[:, HCH : HCH + cw]
                    ki = it_[:, 0:cw]
                    de = ht[:, 0:cw]
                    yt32 = ht[:, HCH : HCH + cw]
                    nc.scalar.activation(y, xf[:, lo:hi], af.Copy, bias=0.0, scale=63.0)
                    nc.scalar.activation(ki, y, af.Copy)                # f32 -> int32
                    nc.scalar.activation(yt, ki, af.Copy)               # int32 -> f32
                    nc.scalar.activation(yt32, ki, af.Copy, bias=-32.0)  # -> fp16, -32
                    nc.vector.tensor_tensor(de, y, yt, alu.is_gt)       # ceil fix bit
                    nc.vector.tensor_tensor(K[:, lo:hi], de, yt32, alu.add)

            # ---- SK0: chi[:, 63] = 32 * sum_slice(K16 owned) ----
            pending = []

            def _emit_tail(pt, ppsum, dest=None):
                d = chi if dest is None else dest
                nc.scalar.activation(
                    scr512[:, :], ppsum[:, :], af.Copy, bias=0.0, scale=32.0,
                    accum_out=d[:, pt : pt + 1],
                )

            spsum = pp.tile([SPC, MCH], dt.float32, tag="ps")
            for ci in range(NMM):
                lo, hi = MCH * ci, min(MCH * (ci + 1), OWN_W)
                nc.tensor.matmul(
                    spsum[:, 0 : hi - lo], bselh[:, 0:64], Ko[:, lo:hi],
                    start=(ci == 0), stop=(ci == NMM - 1),
                )
            pending.append((RES - 1, spsum, chi))

            # ---- w-prep -> V on DVE; zk baseline passes for DVE-thresholds
            #      interleave so PE has colsum work during the whole chain;
            #      first ACT relus also overlap here ----
            with (
                tc.tile_pool(name="zkp", bufs=1) as zp,
                tc.tile_pool(name="wprep", bufs=1) as wp,
            ):
                ehx = wp.tile([NPART, EHX_W], dt.float16)
                tt = wp.tile([NPART, 5 * OWN_W], dt.float16)
                s = [tt[:, i * OWN_W : (i + 1) * OWN_W] for i in range(5)] + [V[:, :]]
                TT = nc.vector.tensor_tensor

                relu_iter = iter(act_list)
                zk_iter = iter(dve_list)
                zct = [0]

                def emit_zk(n=2):
                    for _ in range(n):
                        t = next(zk_iter, None)
                        if t is None:
                            return
                        zct[0] += 1
                        if zct[0] % 2 == 0:
                            rt = next(relu_iter, None)
                            if rt is not None:
                                emit_relu(rt)
                        zk = zp.tile([NPART, OWN_W], dt.float16, tag="zk", bufs=2)
                        nc.vector.tensor_scalar(
                            zk[:, :], Ko, float(t) - 31.5, 1.0, alu.min, alu.mult
                        )
                        psum = pp.tile([SPC, MCH], dt.float32, tag="ps")
                        for ci in range(NMM):
                            lo, hi = MCH * ci, min(MCH * (ci + 1), OWN_W)
                            nc.tensor.matmul(
                                psum[:, 0 : hi - lo], bselh[:, 0:64], zk[:, lo:hi],
                                start=(ci == 0), stop=(ci == NMM - 1),
                            )
                        pending.append((t, psum, zchi))
                        if len(pending) >= 4:
                            _emit_tail(*pending.pop(0))

                TT(ehx[:, :], K[:, 129 : 129 + EHX_W], K[:, 130 : 130 + EHX_W], alu.max)
                emit_zk()
                TT(s[0], Ko, K[:, OWN - 1 : OWN - 1 + OWN_W], alu.is_gt)    # bL'
                emit_zk()
                TT(s[1], Ko, K[:, OWN + 1 : OWN + 1 + OWN_W], alu.is_ge)    # bR'
                emit_zk()
                TT(s[2], Ko, K[:, OWN - SW : OWN - SW + OWN_W], alu.is_gt)  # bU'
                emit_zk()
                TT(s[3], Ko, K[:, OWN + SW : OWN + SW + OWN_W], alu.is_ge)  # bD'
                emit_zk()
                TT(s[4], s[0], s[1], alu.add)                               # e1
                emit_zk()
                TT(s[5], s[2], s[3], alu.add)                               # e2
                emit_zk()
                TT(s[2], s[4], s[5], alu.add)                               # E
                emit_zk()
                TT(s[3], Ko, ehx[:, 0:OWN_W], alu.is_gt)                    # cUL
                emit_zk()
                TT(s[4], s[3], s[0], alu.mult)                              # S_ul
                emit_zk()
                TT(s[5], Ko, ehx[:, 1 : 1 + OWN_W], alu.is_gt)              # cUR
                emit_zk()
                TT(s[3], s[5], s[1], alu.mult)                              # S_ur
                emit_zk()
                TT(s[5], Ko, ehx[:, 260 : 260 + OWN_W], alu.is_ge)          # cLL
                emit_zk()
                TT(s[1], s[5], s[0], alu.mult)                              # S_ll
                emit_zk()
                TT(s[5], Ko, ehx[:, 131 : 131 + OWN_W], alu.is_ge)          # [Ko>=ehx131]
                emit_zk()
                TT(s[0], Ko, ehx[:, 261 : 261 + OWN_W], alu.is_ge)          # [Ko>=ehx261]
                emit_zk()
                TT(s[5], s[5], s[0], alu.mult)                              # S_lr
                emit_zk()
                TT(s[0], s[4], s[3], alu.add)                               # S_ul+S_ur
                emit_zk()
                TT(s[3], s[1], s[5], alu.add)                               # S_ll+S_lr
                emit_zk()
                TT(s[4], s[0], s[3], alu.add)                               # S
                emit_zk()
                TT(s[0], s[4], s[2], alu.subtract)                          # S - E
                emit_zk()
                nc.vector.tensor_scalar(
                    s[1], s[0], 1.0 / 32.0, 1.0 / 32.0, alu.mult, alu.add
                )                                                           # w/32
                TT(V[:, :], s[1], Ko, alu.add)                              # V
                emit_zk(99)

            # ---- threshold loop: V colsum for every t; rest of relus ----
            with tc.tile_pool(name="thr", bufs=4) as mp:
                vpsum = pp.tile([SPC, MCH], dt.float32, tag="ps")
                for ci in range(NMM):
                    lo, hi = MCH * ci, min(MCH * (ci + 1), OWN_W)
                    nc.tensor.matmul(
                        vpsum[:, 0 : hi - lo], bselh[:, 0:64], V[:, lo:hi],
                        start=(ci == 0), stop=(ci == NMM - 1),
                    )
                pending.append((RES - 1, vpsum, zchi))
                for i, t in enumerate(range(63)):
                    if t in af_set:
                        continue
                    ct = float(t) - 31.5
                    zv = mp.tile([NPART, OWN_W], dt.float16, tag="zv", bufs=3)
                    nc.vector.tensor_scalar(
                        zv[:, :], V[:, :], ct, 1.0, alu.min, alu.mult
                    )
                    if i % 3 != 2:
                        rt = next(relu_iter, None)
                        if rt is not None:
                            emit_relu(rt)
                    psum = pp.tile([SPC, MCH], dt.float32, tag="ps")
                    for ci in range(NMM):
                        lo, hi = MCH * ci, min(MCH * (ci + 1), OWN_W)
                        nc.tensor.matmul(
                            psum[:, 0 : hi - lo], bselh[:, 0:64], zv[:, lo:hi],
                            start=(ci == 0), stop=(ci == NMM - 1),
                        )
                    pending.append((t, psum, chi))
                    if len(pending) >= 4:
                        _emit_tail(*pending.pop(0))
                for rt in relu_iter:
                    emit_relu(rt)
                for t in AF_TS:
                    emit_relu_v(t)
                for args in pending:
                    _emit_tail(*args)
                pending.clear()

            nc.vector.memset(zchi[:, 0:1], 0.0)
            nc.sync.dma_start(chi_dram, chi[:, :])
            nc.sync.dma_start(zchi_dram, zchi[:, :])
            nc.sync.dma_start(akr_dram, akr[:, :])
            nc.sync.dma_start(akrv_dram, akrv[:, :])

    if legalize:
        _legalize_waits(nc)
    return nc


def _legalize_waits(nc, max_waits: int = 1):
    """This walrus build rejects instructions with more than one sync wait.
    Split excess waits onto preceding same-engine NoOps."""
    import concourse.mybir as mybir

    for f in nc.m.functions:
        for b in f.blocks:
            il = list(b.instructions)
            out, changed = [], False
            for inst in il:
                try:
                    si = inst.sync_info
                except AttributeError:
                    si = None
                waits = list(si.on_wait) if si else []
                if len(waits) > max_waits:
                    head, keep = waits[:-max_waits], waits[-max_waits:]
                    for k, wv in enumerate(head):
                        out.append(
                            mybir.InstNoOp(
                                name=f"{inst.name}-w{k}",
                                engine=inst.engine,
                                sync_info=mybir.SyncInfo(on_wait=[wv], on_update=[]),
                                bass_nofuse=True,
                            )
                        )
                    inst.sync_info = mybir.SyncInfo(
                        on_wait=keep, on_update=list(si.on_update)
                    )
                    changed = True
                out.append(inst)
            if changed:
                b.instructions = out


def make_host_inputs(xcore: np.ndarray):
    """xcore [SPC, H, W] f32 -> packed xi [NPART, WTOT]."""
    xi = np.full((SPC, 2, ROWS, SW), XSENT, dtype=np.float32)
    xh = xcore.reshape(SPC, 2, 64, W)
    xi[:, :, 2:66, 0:W] = xh
    xi[:, 1, 1, 0:W] = xcore[:, 63, :]   # h=1 up-overlap = image row 63
    xi[:, 0, 66, 0:W] = xcore[:, 64, :]  # h=0 down-overlap = image row 64
    return xi.reshape(NPART, WTOT)


def _host_bsel_bias():
    bsel = np.zeros((NPART, 128), dtype=np.float32)
    bsel[np.arange(NPART), np.arange(NPART) // 2] = 1.0
    bsel[np.arange(NPART), 64 + np.arange(NPART) // 2] = -1.0
    # ACT relu bias: Relu(K16 + bias) with bias = -c_t = 31.5 - t
    bias = np.broadcast_to(
        31.5 - np.arange(64, dtype=np.float32), (NPART, 64)
    ).copy()
    return bsel, bias


def _install_ntff_hook():
    import sys, types

    if "antenv.axon_hooks" in sys.modules:
        return
    mod = types.ModuleType("antenv.axon_hooks")
    state = {"hook": None}
    mod.set_axon_ntff_profile_hook = lambda h: state.update(hook=h)
    mod.get_axon_ntff_profile_hook = lambda: state["hook"]
    sys.modules["antenv.axon_hooks"] = mod
    try:
        from trn_agent_boot.trn_boot import _ntff_profile_via_ctypes

        hook = _ntff_profile_via_ctypes("/opt/axon/libaxon_pjrt.so")
        if hook is not None:
            mod.set_axon_ntff_profile_hook(hook)
    except Exception:
        pass


def _run(x: np.ndarray, trace: bool = False):
    from concourse import bass_utils

    if trace:
        _install_ntff_hook()

    x = np.ascontiguousarray(np.asarray(x), dtype=np.float32)
    assert x.shape == (B, C, H, W)

    if "nc" not in _CACHE:
        _CACHE["nc"] = _build_program()
    nc = _CACHE["nc"]

    bsel, bias = _host_bsel_bias()
    flat = x.reshape(SLICES, H, W)
    in_maps = []
    for k in range(NCORES):
        xi = make_host_inputs(flat[k * SPC : (k + 1) * SPC])
        in_maps.append({"xi": xi, "bsel": bsel, "bias": bias})
    res = bass_utils.run_bass_kernel_spmd(
        nc, in_maps, core_ids=list(range(NCORES)), trace=trace
    )

    ecc = np.empty((SLICES, RES), dtype=np.float64)
    act_set = set(ACT_TS)
    af_set = set(AF_TS)
    for k in range(NCORES):
        chi = res.results[k]["chi"].astype(np.float64)   # [SPC, 64]
        zchi = res.results[k]["zchi"].astype(np.float64)  # [SPC, 64]
        akr = res.results[k]["akr"].astype(np.float64)   # [NPART, 64]
        akrv = res.results[k]["akrv"].astype(np.float64)  # [NPART, 64]
        akr_s = akr.reshape(SPC, 2, 64).sum(axis=1)      # per-slice relu sums
        akrv_s = akrv.reshape(SPC, 2, 64).sum(axis=1)
        sl = slice(k * SPC, (k + 1) * SPC)
        for t in range(RES - 1):
            if t in af_set:
                # 32*minV = zchi63 - 32*rv ; 32*minK = chi63 - 32*rk
                ecc[sl, t] = (zchi[:, 63] - chi[:, 63]
                              + 32.0 * (akr_s[:, t] - akrv_s[:, t]))
            elif t in act_set:
                # chi_col = 32*sum(min(V,c)); chi[:,63] = 32*SK0;
                # 32*sum(min(K16,c)) = chi63 - 32*relu_sum
                ecc[sl, t] = chi[:, t] - chi[:, 63] + 32.0 * akr_s[:, t]
            else:
                ecc[sl, t] = chi[:, t] - zchi[:, t]
        ecc[:, RES - 1] = 1.0
    return ecc.reshape(B, C, RES).astype(np.float32), res


def kernel(x: np.ndarray) -> np.ndarray:
    out, _ = _run(x, trace=False)
    return out


# revision 6
# speedup vs baseline: 1.1688x; 1.1688x over previous
"""Euler characteristic curve (cubical complex) kernel for Trainium2.

Problem: x [32,16,128,128] f32 -> ECC [32,16,64] f32.
Per (b,c) slice: every cell of the 255x255 vertex-mode cubical grid has
filtration bin K = ceil(63*max(corner values)) in [0,63];
ECC(t) = #V(K<=t) - #Eh(K<=t) - #Ev(K<=t) + #Q(K<=t).

Strategy (per core, 64 slices, pure data parallel over 8 cores):
 - Lower-star compression: chi(t) = sum_v w_v * [K_v <= t] with integer
   vertex weight w_v = 1 - (#edges assigned) + (#squares assigned).
 - Min-clamp difference trick (no per-threshold mask*weight multiply):
   encode V = K16 + w/32 (fp16-exact: |K16|<=32 needs 6 bits + 5
   fraction bits, K16 = K - 32). With cut c_t = t - 31.5:
       sum_f [min(V,c_t) - min(K16,c_t)] = (1/32) * sum_v w_v [K_v<=t]
   exactly (K=t kept since V <= K16+1/32 < c_t+...; K=t+1 clamped since
   V >= K16-3/32 > c_t; sentinels clamp on both sides and cancel).
 - Per threshold: one DVE tensor_scalar (min,mult) pass at 4x producing
   zV; the K16 baseline either as a second DVE min pass (PE subtracts it
   via a negated bsel column block accumulated into the same PSUM) or on
   ACT as Relu(K16 - c_t)+accum (sum min = SK0 - sum relu), chosen per
   threshold to balance engines. PE column-sum matmuls with +/-1 slice-
   selector weights, ACT Copy(scale=32)+accum tail -> chi[:, t].
 - chi[:, 63] holds 32*SK0 per slice (t=63 output is the host constant 1).
 - All arithmetic exact in integers/32nds; rel err 0.
"""

import numpy as np

B, C, H, W = 32, 16, 128, 128
RES = 64
NCORES = 8
SLICES = B * C              # 512
SPC = SLICES // NCORES      # 64 slices per core
NPART = 128

SW = 130                    # row stride: 128 cols + 2 sentinel columns
ROWS = 67                   # pad row + up-overlap + 64 owned + down-overlap
WTOT = ROWS * SW            # 8710 input width per partition
KW = WTOT + 4               # K tile width (pad, memset to sentinel)
OWN = 260                   # owned rows start (flat offset, row 2)
OWN_W = 64 * SW             # 8320 owned width
EHX_W = 8582                # ehx width (Eh over flat 129..8711)
XSENT = 20.25               # x sentinel -> K = 1276, K16 = 1244
SENT16 = 1244.0
MCH = 256                   # PE chunk width
NMM = (OWN_W + MCH - 1) // MCH

# thresholds whose K16-baseline runs on ACT (Relu+accum); rest on DVE.
# ~35 of 63 balances ACT vs DVE/PE; first 12 emitted during w-prep.


def _act_ts(k=35):
    """Evenly spread k ACT-baseline thresholds among 0..62."""
    idx = np.linspace(0, 62, k).round().astype(int)
    return tuple(sorted(set(idx.tolist())))


ACT_TS = _act_ts(36)
AF_TS = ACT_TS[2::9][:4]            # ~8 ACT-full thresholds (relu-V too)

_CACHE = {}


def _build_program(legalize=True):
    import concourse.bass as bass
    import concourse.mybir as mybir
    from concourse.tile import TileContext
    from concourse.alu_op_type import AluOpType as alu

    dt = mybir.dt
    af = mybir.ActivationFunctionType
    nc = bass.Bass("TRN2", target_bir_lowering=False, debug=False)

    x_dram = nc.dram_tensor("xi", [NPART, WTOT], dt.float32, kind="ExternalInput").ap()
    bsel_dram = nc.dram_tensor("bsel", [NPART, 128], dt.float32, kind="ExternalInput").ap()
    bias_dram = nc.dram_tensor("bias", [NPART, 64], dt.float32, kind="ExternalInput").ap()
    chi_dram = nc.dram_tensor("chi", [SPC, 64], dt.float32, kind="ExternalOutput").ap()
    akr_dram = nc.dram_tensor("akr", [NPART, 64], dt.float32, kind="ExternalOutput").ap()
    zchi_dram = nc.dram_tensor("zchi", [SPC, 64], dt.float32, kind="ExternalOutput").ap()
    akrv_dram = nc.dram_tensor("akrv", [NPART, 64], dt.float32, kind="ExternalOutput").ap()

    HCH = 2178  # K-compute column chunk width (4 chunks, last 2176)

    act_list = list(ACT_TS)
    af_set = set(AF_TS)
    dve_list = [t for t in range(63) if t not in ACT_TS]

    with TileContext(nc) as tc:
        with (
            tc.tile_pool(name="persist", bufs=1) as ap_,
            tc.tile_pool(name="ps", bufs=8, space="PSUM") as pp,
        ):
            # ---- persistent tiles ----
            K = ap_.tile([NPART, KW], dt.float16)     # K16 = ceil(63x) - 32
            V = ap_.tile([NPART, OWN_W], dt.float16)  # K16 + w/32
            bself = ap_.tile([NPART, 128], dt.float32)
            bselh = ap_.tile([NPART, 128], dt.float16)
            biasT = ap_.tile([NPART, 64], dt.float32)
            chi = ap_.tile([SPC, 64], dt.float32)
            zchi = ap_.tile([SPC, 64], dt.float32)
            akr = ap_.tile([NPART, 64], dt.float32)
            akrv = ap_.tile([NPART, 64], dt.float32)
            za = ap_.tile([NPART, OWN_W], dt.float8e4)  # ACT relu scratch

            nc.sync.dma_start(bself[:, :], bsel_dram)
            nc.sync.dma_start(biasT[:, :], bias_dram)
            nc.vector.tensor_copy(bselh[:, :], bself[:, :])
            nc.vector.memset(K[:, WTOT:KW], SENT16)
            nc.vector.memset(akr[:, :], 0.0)
            nc.vector.memset(akrv[:, :], 0.0)

            Ko = K[:, OWN : OWN + OWN_W]

            def emit_relu(t):
                nc.scalar.activation(
                    za[:, :], Ko, af.Relu, bias=biasT[:, t : t + 1], scale=1.0,
                    accum_out=akr[:, t : t + 1],
                )

            def emit_relu_v(t):
                nc.scalar.activation(
                    za[:, :], V[:, :], af.Relu, bias=biasT[:, t : t + 1], scale=1.0,
                    accum_out=akrv[:, t : t + 1],
                )

            # ---- K16 = ceil(63*x) - 32 ----
            with tc.tile_pool(name="kprep", bufs=1) as kp:
                xf = kp.tile([NPART, WTOT], dt.float32)
                ft = kp.tile([NPART, 2 * HCH], dt.float32)
                it_ = kp.tile([NPART, HCH], dt.int32)
                ht = kp.tile([NPART, 2 * HCH], dt.float16)
                chunks = [(i * HCH, min((i + 1) * HCH, WTOT)) for i in range(4)]
                for lo, hi in chunks:
                    nc.sync.dma_start(xf[:, lo:hi], x_dram[:, lo:hi])
                for lo, hi in chunks:
                    cw = hi - lo
                    y = ft[:, 0:cw]
                    yt = ft[:, HCH : HCH + cw]
                    ki = it_[:, 0:cw]
                    de = ht[:, 0:cw]
                    yt32 = ht[:, HCH : HCH + cw]
                    nc.scalar.activation(y, xf[:, lo:hi], af.Copy, bias=0.0, scale=63.0)
                    nc.scalar.activation(ki, y, af.Copy)                # f32 -> int32
                    nc.scalar.activation(yt, ki, af.Copy)               # int32 -> f32
                    nc.scalar.activation(yt32, ki, af.Copy, bias=-32.0)  # -> fp16, -32
                    nc.vector.tensor_tensor(de, y, yt, alu.is_gt)       # ceil fix bit
                    nc.vector.tensor_tensor(K[:, lo:hi], de, yt32, alu.add)

            # ---- SK0: chi[:, 63] = 32 * sum_slice(K16 owned) ----
            pending = []

            def _emit_tail(pt, ppsum, dest=None):
                d = chi if dest is None else dest
                nc.vector.tensor_reduce(
                    d[:, pt : pt + 1], ppsum[:, :], mybir.AxisListType.X, alu.add
                )

            spsum = pp.tile([SPC, MCH], dt.float32, tag="ps")
            for ci in range(NMM):
                lo, hi = MCH * ci, min(MCH * (ci + 1), OWN_W)
                nc.tensor.matmul(
                    spsum[:, 0 : hi - lo], bselh[:, 0:64], Ko[:, lo:hi],
                    start=(ci == 0), stop=(ci == NMM - 1),
                )
            pending.append((RES - 1, spsum, chi))

            # ---- w-prep -> V on DVE; zk baseline passes for DVE-thresholds
            #      interleave so PE has colsum work during the whole chain;
            #      first ACT relus also overlap here ----
            with (
                tc.tile_pool(name="zkp", bufs=1) as zp,
                tc.tile_pool(name="wprep", bufs=1) as wp,
            ):
                ehx = wp.tile([NPART, EHX_W], dt.float16)
                tt = wp.tile([NPART, 5 * OWN_W], dt.float16)
                s = [tt[:, i * OWN_W : (i + 1) * OWN_W] for i in range(5)] + [V[:, :]]
                TT = nc.vector.tensor_tensor

                relu_iter = iter(act_list)
                zk_iter = iter(dve_list)
                zct = [0]

                def emit_zk(n=2):
                    for _ in range(n):
                        t = next(zk_iter, None)
                        if t is None:
                            return
                        zct[0] += 1
                        if zct[0] % 2 == 0:
                            rt = next(relu_iter, None)
                            if rt is not None:
                                emit_relu(rt)
                        zk = zp.tile([NPART, OWN_W], dt.float16, tag="zk", bufs=2)
                        nc.vector.tensor_scalar(
                            zk[:, :], Ko, float(t) - 31.5, 1.0, alu.min, alu.mult
                        )
                        psum = pp.tile([SPC, MCH], dt.float32, tag="ps")
                        for ci in range(NMM):
                            lo, hi = MCH * ci, min(MCH * (ci + 1), OWN_W)
                            nc.tensor.matmul(
                                psum[:, 0 : hi - lo], bselh[:, 0:64], zk[:, lo:hi],
                                start=(ci == 0), stop=(ci == NMM - 1),
                            )
                        pending.append((t, psum, zchi))
                        if len(pending) >= 4:
                            _emit_tail(*pending.pop(0))

                TT(ehx[:, :], K[:, 129 : 129 + EHX_W], K[:, 130 : 130 + EHX_W], alu.max)
                emit_zk()
                TT(s[0], Ko, K[:, OWN - 1 : OWN - 1 + OWN_W], alu.is_gt)    # bL'
                emit_zk()
                TT(s[1], Ko, K[:, OWN + 1 : OWN + 1 + OWN_W], alu.is_ge)    # bR'
                emit_zk()
                TT(s[2], Ko, K[:, OWN - SW : OWN - SW + OWN_W], alu.is_gt)  # bU'
                emit_zk()
                TT(s[3], Ko, K[:, OWN + SW : OWN + SW + OWN_W], alu.is_ge)  # bD'
                emit_zk()
                TT(s[4], s[0], s[1], alu.add)                               # e1
                emit_zk()
                TT(s[5], s[2], s[3], alu.add)                               # e2
                emit_zk()
                TT(s[2], s[4], s[5], alu.add)                               # E
                emit_zk()
                TT(s[3], Ko, ehx[:, 0:OWN_W], alu.is_gt)                    # cUL
                emit_zk()
                TT(s[4], s[3], s[0], alu.mult)                              # S_ul
                emit_zk()
                TT(s[5], Ko, ehx[:, 1 : 1 + OWN_W], alu.is_gt)              # cUR
                emit_zk()
                TT(s[3], s[5], s[1], alu.mult)                              # S_ur
                emit_zk()
                TT(s[5], Ko, ehx[:, 260 : 260 + OWN_W], alu.is_ge)          # cLL
                emit_zk()
                TT(s[1], s[5], s[0], alu.mult)                              # S_ll
                emit_zk()
                TT(s[5], Ko, ehx[:, 131 : 131 + OWN_W], alu.is_ge)          # [Ko>=ehx131]
                emit_zk()
                TT(s[0], Ko, ehx[:, 261 : 261 + OWN_W], alu.is_ge)          # [Ko>=ehx261]
                emit_zk()
                TT(s[5], s[5], s[0], alu.mult)                              # S_lr
                emit_zk()
                TT(s[0], s[4], s[3], alu.add)                               # S_ul+S_ur
                emit_zk()
                TT(s[3], s[1], s[5], alu.add)                               # S_ll+S_lr
                emit_zk()
                TT(s[4], s[0], s[3], alu.add)                               # S
                emit_zk()
                TT(s[0], s[4], s[2], alu.subtract)                          # S - E
                emit_zk()
                nc.vector.tensor_scalar(
                    s[1], s[0], 1.0 / 32.0, 1.0 / 32.0, alu.mult, alu.add
                )                                                           # w/32
                TT(V[:, :], s[1], Ko, alu.add)                              # V
                emit_zk(99)

            # ---- threshold loop: V colsum for every t; rest of relus ----
            with tc.tile_pool(name="thr", bufs=4) as mp:
                vpsum = pp.tile([SPC, MCH], dt.float32, tag="ps")
                for ci in range(NMM):
                    lo, hi = MCH * ci, min(MCH * (ci + 1), OWN_W)
                    nc.tensor.matmul(
                        vpsum[:, 0 : hi - lo], bselh[:, 0:64], V[:, lo:hi],
                        start=(ci == 0), stop=(ci == NMM - 1),
                    )
                pending.append((RES - 1, vpsum, zchi))
                for i, t in enumerate(range(63)):
                    if t in af_set:
                        continue
                    ct = float(t) - 31.5
                    zv = mp.tile([NPART, OWN_W], dt.float16, tag="zv", bufs=3)
                    nc.vector.tensor_scalar(
                        zv[:, :], V[:, :], ct, 1.0, alu.min, alu.mult
                    )
                    if i % 3 != 2:
                        rt = next(relu_iter, None)
                        if rt is not None:
                            emit_relu(rt)
                    psum = pp.tile([SPC, MCH], dt.float32, tag="ps")
                    for ci in range(NMM):
                        lo, hi = MCH * ci, min(MCH * (ci + 1), OWN_W)
                        nc.tensor.matmul(
                            psum[:, 0 : hi - lo], bselh[:, 0:64], zv[:, lo:hi],
                            start=(ci == 0), stop=(ci == NMM - 1),
                        )
                    pending.append((t, psum, chi))
                    if len(pending) >= 4:
                        _emit_tail(*pending.pop(0))
                for rt in relu_iter:
                    emit_relu(rt)
                for t in AF_TS:
                    emit_relu_v(t)
                for args in pending:
                    _emit_tail(*args)
                pending.clear()

            nc.vector.memset(zchi[:, 0:1], 0.0)
            nc.sync.dma_start(chi_dram, chi[:, :])
            nc.sync.dma_start(zchi_dram, zchi[:, :])
            nc.sync.dma_start(akr_dram, akr[:, :])
            nc.sync.dma_start(akrv_dram, akrv[:, :])

    if legalize:
        _legalize_waits(nc)
    return nc


def _legalize_waits(nc, max_waits: int = 1):
    """This walrus build rejects instructions with more than one sync wait.
    Split excess waits onto preceding same-engine NoOps."""
    import concourse.mybir as mybir

    for f in nc.m.functions:
        for b in f.blocks:
            il = list(b.instructions)
            out, changed = [], False
            for inst in il:
                try:
                    si = inst.sync_info
                except AttributeError:
                    si = None
                waits = list(si.on_wait) if si else []
                if len(waits) > max_waits:
                    head, keep = waits[:-max_waits], waits[-max_waits:]
                    for k, wv in enumerate(head):
                        out.append(
                            mybir.InstNoOp(
                                name=f"{inst.name}-w{k}",
                                engine=inst.engine,
                                sync_info=mybir.SyncInfo(on_wait=[wv], on_update=[]),
                                bass_nofuse=True,
                            )
                        )
                    inst.sync_info = mybir.SyncInfo(
                        on_wait=keep, on_update=list(si.on_update)
                    )
                    changed = True
                out.append(inst)
            if changed:
                b.instructions = out


def make_host_inputs(xcore: np.ndarray):
    """xcore [SPC, H, W] f32 -> packed xi [NPART, WTOT]."""
    xi = np.full((SPC, 2, ROWS, SW), XSENT, dtype=np.float32)
    xh = xcore.reshape(SPC, 2, 64, W)
    xi[:, :, 2:66, 0:W] = xh
    xi[:, 1, 1, 0:W] = xcore[:, 63, :]   # h=1 up-overlap = image row 63
    xi[:, 0, 66, 0:W] = xcore[:, 64, :]  # h=0 down-overlap = image row 64
    return xi.reshape(NPART, WTOT)


def _host_bsel_bias():
    bsel = np.zeros((NPART, 128), dtype=np.float32)
    bsel[np.arange(NPART), np.arange(NPART) // 2] = 32.0
    bsel[np.arange(NPART), 64 + np.arange(NPART) // 2] = -32.0
    # ACT relu bias: Relu(K16 + bias) with bias = -c_t = 31.5 - t
    bias = np.broadcast_to(
        31.5 - np.arange(64, dtype=np.float32), (NPART, 64)
    ).copy()
    return bsel, bias


def _install_ntff_hook():
    import sys, types

    if "antenv.axon_hooks" in sys.modules:
        return
    mod = types.ModuleType("antenv.axon_hooks")
    state = {"hook": None}
    mod.set_axon_ntff_profile_hook = lambda h: state.update(hook=h)
    mod.get_axon_ntff_profile_hook = lambda: state["hook"]
    sys.modules["antenv.axon_hooks"] = mod
    try:
        from trn_agent_boot.trn_boot import _ntff_profile_via_ctypes

        hook = _ntff_profile_via_ctypes("/opt/axon/libaxon_pjrt.so")
        if hook is not None:
            mod.set_axon_ntff_profile_hook(hook)
    except Exception:
        pass


def _run(x: np.ndarray, trace: bool = False):
    from concourse import bass_utils

    if trace:
        _install_ntff_hook()

    x = np.ascontiguousarray(np.asarray(x), dtype=np.float32)
    assert x.shape == (B, C, H, W)

    if "nc" not in _CACHE:
        _CACHE["nc"] = _build_program()
    nc = _CACHE["nc"]

    bsel, bias = _host_bsel_bias()
    flat = x.reshape(SLICES, H, W)
    in_maps = []
    for k in range(NCORES):
        xi = make_host_inputs(flat[k * SPC : (k + 1) * SPC])
        in_maps.append({"xi": xi, "bsel": bsel, "bias": bias})
    res = bass_utils.run_bass_kernel_spmd(
        nc, in_maps, core_ids=list(range(NCORES)), trace=trace
    )

    ecc = np.empty((SLICES, RES), dtype=np.float64)
    act_set = set(ACT_TS)
    af_set = set(AF_TS)
    for k in range(NCORES):
        chi = res.results[k]["chi"].astype(np.float64)   # [SPC, 64]
        zchi = res.results[k]["zchi"].astype(np.float64)  # [SPC, 64]
        akr = res.results[k]["akr"].astype(np.float64)   # [NPART, 64]
        akrv = res.results[k]["akrv"].astype(np.float64)  # [NPART, 64]
        akr_s = akr.reshape(SPC, 2, 64).sum(axis=1)      # per-slice relu sums
        akrv_s = akrv.reshape(SPC, 2, 64).sum(axis=1)
        sl = slice(k * SPC, (k + 1) * SPC)
        for t in range(RES - 1):
            if t in af_set:
                # 32*minV = zchi63 - 32*rv ; 32*minK = chi63 - 32*rk
                ecc[sl, t] = (zchi[:, 63] - chi[:, 63]
                              + 32.0 * (akr_s[:, t] - akrv_s[:, t]))
            elif t in act_set:
                # chi_col = 32*sum(min(V,c)); chi[:,63] = 32*SK0;
                # 32*sum(min(K16,c)) = chi63 - 32*relu_sum
                ecc[sl, t] = chi[:, t] - chi[:, 63] + 32.0 * akr_s[:, t]
            else:
                ecc[sl, t] = chi[:, t] - zchi[:, t]
        ecc[:, RES - 1] = 1.0
    return ecc.reshape(B, C, RES).astype(np.float32), res


def kernel(x: np.ndarray) -> np.ndarray:
    out, _ = _run(x, trace=False)
    return out
